# revision 69
# baseline (speedup 1.0000x reference)
"""Trainium2 Bass kernel for nn_AttentionLayer (cross-attention + FF + LayerNorm).

Strategy (data-parallel over batch, 2 batch elements per core, no collectives):
  - fp8e4m3 DoubleRow matmuls (256-deep contraction, 0.5 cyc/row in the
    TimelineSim cost model, i.e. 4x cheaper than bf16 per unit contraction)
    for the Q/K/V projections and the scores matmul, with error-compensated
    "split fp8": text/image ship as fp8-high + fp8-residual pairs
    (interleaved per chunk in one tensor: [.., chunk, 2, len]); weights are
    scaled x32 host-side (uniform(+-1/sqrt(fanin)) weights sit in fp8's
    subnormal range unscaled), un-scaled for free via the Act writeback
    scale=1/32. Q/K/V all run 2 passes W(Xh+Xl) with single-fp8 weights
    (the weight-residual passes were dropped one by one against the
    measured error budget). Q/K are stored fp8 so scores is one DoubleRow
    matmul per token chunk. HW end-to-end rel err 1.75e-2 vs the 2e-2
    gate (measured on the real cores; deterministic).
  - The softmax denominator uses a DoubleRow ones-matmul over an fp8 copy
    of e made on the (otherwise idle) GPSIMD; the 64-token tail chunk sums
    the bf16 e directly so pd never waits on the last copy. The reciprocal
    is broadcast across partitions by GPSIMD partition_broadcast (replaces
    a PE ones-matmul + Act copy). e / V / px stay bf16, reversion (wr, xf)
    and the residual/LN path stay f32r, so the dominant error terms stay
    high precision.
  - Inputs loaded FEATURE-major as pre-transposed fp8 from the host (plain
    contiguous DMAs, no XBAR transpose; same bytes as bf16). Few, large
    DMAs: the HWDGE serializes ~625ns per DMA instruction and each DMA
    dependency edge pays a ~900ns sem-prop, so instruction count matters
    more than piece size. Startup orders the queue by pair-0's consumption
    and interleaves K/Q chunk-group matmuls with the piece arrivals; wr
    (6.3MB) is spread over pairs 1-8 so per-pair weight loads never queue
    behind it; weights prefetch 2 pairs ahead.
  - Per pair: scores and V chunks interleave (the exp Acts drain score
    PSUM banks slower than the PE can issue, V chunks cover the recycle);
    the softmax tail of pair i-1 (pd, recip, broadcast, V^T@E) is emitted
    inside pair i's projections so PE never stalls on Act/DVE round trips.
  - Softmax un-normalized in [key, query] layout; normalization folded
    into the PSUM->SBUF move of x (TT mult with the broadcast reciprocal).
  - FF2 residual+bias: even chunks fused into one scalar_tensor_tensor
    (DVE), odd chunks accumulate the residual on the PE via an identity
    matmul with Act writeback; batch 0's FF1->relu->FF2 round trip is
    covered by pair-9's K matmuls; LayerNorm stats as ones-matmuls
    interleaved behind FF2, transposed to per-token columns via
    16-row-replicated PE transposes; rstd via quake bit-trick + one Newton
    step (DVE only); normalize fused into the PSUM->SBUF copies after the
    transpose back to token-major (Act/DVE halves); one whole-chunk output
    DMA per (batch, token-chunk), batch-1 tail chunks on distinct y tiles
    so the final stores pipeline.
  - V bias folded into the reversion bias host-side: breff = br + bv @ wr.
  - gamma/beta are applied only when not identity (build-time
    specialization keyed on the actual input values).

TimelineSim: 215.3 us (prior session's bf16 kernel: 344.4 us; original
baseline: 515 us). PE busy ~192 us (~89%). HW rel err 1.753e-2 (PASS).
"""

import os
import sys

import numpy as np

# ---- problem constants (hardcoded per contract) ----
B_TOTAL = 16
N_CORES = 8
B = B_TOTAL // N_CORES  # per-core batch
LT, DT = 512, 768       # text tokens / dim
LI, DI = 576, 1024      # image tokens / dim
H, NH, HD = 2048, 8, 256
FF = 128
ISCALE = 1.0 / 16.0     # 1/sqrt(HD)
NPAIR = B * NH          # 16 (batch, head) pairs per core
ITC = [(t, 128 if t < 4 else LI - 512) for t in range(5)]  # image tok chunks
NCD = DT // 128         # 6
NCI = DI // 128         # 8
NT = LT // 128          # 4
WS = 32.0               # host-side fp8 weight pre-scale
RS = 1.0 / WS

_BUILD_CACHE: dict = {}


def _ensure_import_path():
    try:
        import concourse  # noqa: F401
    except ModuleNotFoundError:
        for p in ("/opt/trn_rl_repo", "/root/.axon_site/_ro/trn_rl_repo"):
            if os.path.isdir(p) and p not in sys.path:
                sys.path.insert(0, p)


def build_module(apply_gamma: bool = False):
    key = ("v4", apply_gamma)
    if key in _BUILD_CACHE:
        return _BUILD_CACHE[key]
    _ensure_import_path()
    from contextlib import ExitStack

    import concourse.bacc as bacc
    import concourse.bass as bass  # noqa: F401
    import concourse.mybir as mybir
    import concourse.tile as tile
    from concourse.masks import make_identity

    f32 = mybir.dt.float32
    f32r = mybir.dt.float32r
    bf16 = mybir.dt.bfloat16
    f8 = mybir.dt.float8e4
    AF = mybir.ActivationFunctionType
    ALU = mybir.AluOpType
    DRow = mybir.MatmulPerfMode.DoubleRow

    def r(ap):
        return ap.bitcast(f32r)

    nc = bacc.Bacc("TRN2", target_bir_lowering=False, debug=False, num_devices=N_CORES)

    # text/image ship as [.., chunk, 2, len] with the fp8 high/low split
    # parts interleaved per chunk: one big DMA per tensor (fewer HWDGE slots
    # and 900ns DMA-sem edges), and the DoubleRow slices address either part
    t2p = nc.dram_tensor("t2p", [B, 128, NCD, 2, LT], f8,
                         kind="ExternalInput").ap()
    i2p = nc.dram_tensor("i2p", [B, 128, NCI, 2, LI], f8,
                         kind="ExternalInput").ap()
    wqph = nc.dram_tensor("wqph", [128, NH, NCD, HD], f8,
                          kind="ExternalInput").ap()
    wkph = nc.dram_tensor("wkph", [128, NH, NCI, HD], f8,
                          kind="ExternalInput").ap()
    wvph = nc.dram_tensor("wvph", [128, NH, NCI, HD], f8,
                          kind="ExternalInput").ap()
    wrp = nc.dram_tensor("wrp", [128, H // 128, DT], f32,
                         kind="ExternalInput").ap()
    w1p = nc.dram_tensor("w1p", [128, NCD, FF], bf16, kind="ExternalInput").ap()
    w2 = nc.dram_tensor("w2", [FF, DT], bf16, kind="ExternalInput").ap()
    bqp = nc.dram_tensor("bqp", [128, H // 128], f32, kind="ExternalInput").ap()
    bkp = nc.dram_tensor("bkp", [128, H // 128], f32, kind="ExternalInput").ap()
    b1p = nc.dram_tensor("b1p", [128, 1], f32, kind="ExternalInput").ap()
    b2p = nc.dram_tensor("b2p", [128, NCD], f32, kind="ExternalInput").ap()
    breffp = nc.dram_tensor("breffp", [128, NCD], f32,
                            kind="ExternalInput").ap()
    gamp = nc.dram_tensor("gamp", [128, DT], f32, kind="ExternalInput").ap()
    betp = nc.dram_tensor("betp", [128, DT], f32, kind="ExternalInput").ap()
    out = nc.dram_tensor("out", [B, LT, DT], f32, kind="ExternalOutput").ap()

    with tile.TileContext(nc) as tc, ExitStack() as ctx:
        const = ctx.enter_context(tc.tile_pool(name="const", bufs=1))
        ident = const.tile([128, 128], f32)
        make_identity(nc, ident)
        # the copy performs the required fp32 -> fp32r rounding for the
        # f32r matmul consumers (BIR verifier rejects unrounded producers)
        ident_r = const.tile([128, 128], f32)
        nc.vector.tensor_copy(out=r(ident_r), in_=ident)
        ones_col_b = const.tile([128, 1], bf16)
        nc.vector.memset(ones_col_b, 1.0)
        # fp8 ones for the DoubleRow softmax-denominator matmul; middle-dim
        # stride kept at 16B for the DR weight AP alignment requirement
        ones8_pd = const.tile([128, 2, 16], f8)
        nc.vector.memset(ones8_pd, 1.0)
        ones_row_b = const.tile([1, 128], bf16)
        nc.vector.memset(ones_row_b, 1.0)
        ones_tmp = const.tile([128, 16], f32)
        nc.vector.memset(ones_tmp, 1.0)
        ones16_f = const.tile([128, 16], f32)
        nc.vector.tensor_copy(out=r(ones16_f), in_=ones_tmp)
        ones16_b = const.tile([128, 16], bf16)
        nc.vector.memset(ones16_b, 1.0)
        # float with bit pattern 0x5f3759df (quake rsqrt magic)
        magic4 = const.tile([128, NT], f32)
        nc.vector.memset(magic4, float(np.uint32(0x5F3759DF).view(np.float32)))
        # dummy activation: forces the act-table load off the critical path
        warm = const.tile([1, 1], f32)
        nc.scalar.activation(out=warm, in_=magic4[0:1, 0:1], func=AF.Exp,
                             scale=0.0)

        bq_sb = const.tile([128, H // 128], f32)
        bk_sb = const.tile([128, H // 128], f32)
        b1_sb = const.tile([128, 1], f32)
        b2_sb = const.tile([128, NCD], f32)
        breff_sb = const.tile([128, NCD], f32)
        w1_sb = const.tile([128, NCD, FF], bf16)
        w2_sb = const.tile([128, DT], bf16)
        wr_sb = const.tile([128, H // 128, DT], f32)  # loaded in 4 chunks mid-flight
        gam_sb = bet_sb = None
        if apply_gamma:
            gam_sb = const.tile([128, DT], f32)
            bet_sb = const.tile([128, DT], f32)

        def load_consts_late():
            nc.sync.dma_start(out=b1_sb, in_=b1p)
            nc.sync.dma_start(out=b2_sb, in_=b2p)
            nc.sync.dma_start(out=breff_sb, in_=breffp)
            nc.sync.dma_start(out=w1_sb, in_=w1p)
            nc.sync.dma_start(out=w2_sb, in_=w2)
            if apply_gamma:
                nc.sync.dma_start(out=gam_sb, in_=gamp)
                nc.sync.dma_start(out=bet_sb, in_=betp)

        psum = ctx.enter_context(tc.tile_pool(name="psum", bufs=6, space="PSUM"))
        psd = ctx.enter_context(tc.tile_pool(name="psd", bufs=2, space="PSUM"))
        tfp = ctx.enter_context(tc.tile_pool(name="tfp", bufs=2))
        hwp = ctx.enter_context(tc.tile_pool(name="hwp", bufs=3))
        atp = ctx.enter_context(tc.tile_pool(name="atp", bufs=2))
        xfp = ctx.enter_context(tc.tile_pool(name="xfp", bufs=1))
        ofp = ctx.enter_context(tc.tile_pool(name="ofp", bufs=1))
        p5p = ctx.enter_context(tc.tile_pool(name="p5p", bufs=2))

        TFI: dict = {}
        W: dict = {}
        S: dict = {}
        XF: dict = {}
        OF: dict = {}
        SQ: dict = {}

        def load_weights(i, eng=None):
            eng = eng or nc.sync
            h = i % NH
            wq_hh = hwp.tile([128, NCD, HD], f8, tag="wqhh", name="wqhh")
            eng.dma_start(out=wq_hh, in_=wqph[:, h])
            wk_hh = hwp.tile([128, NCI, HD], f8, tag="wkhh", name="wkhh")
            eng.dma_start(out=wk_hh, in_=wkph[:, h])
            wv_hh = hwp.tile([128, NCI, HD], f8, tag="wvhh", name="wvhh")
            eng.dma_start(out=wv_hh, in_=wvph[:, h])
            W[i] = (wq_hh, wk_hh, wv_hh)

        def load_wr_chunk(cc):
            # wr is 6.3MB (~19us of bus): spread it thinly (8 chunks over
            # pairs 1-8) so the SP queue's per-pair weight loads never sit
            # behind a multi-us transfer
            nc.sync.dma_start(
                out=r(wr_sb[:, cc * 2:(cc + 1) * 2, :]),
                in_=r(wrp[:, cc * 2:(cc + 1) * 2, :]))

        def alloc_xf(b):
            XF[b] = [xfp.tile([128, LT], f32, tag=f"xf{c}", name=f"xf{c}")
                     for c in range(H // 128)]

        def emit_Q(i):
            b, h = divmod(i, NH)
            tf2 = TFI[b][0]
            wq_hh = W[i][0]
            q_h = atp.tile([128, 2, LT], f8, tag="qh", name="qh")
            NP = NCD // 2
            NMM = 2 * NP
            for m in range(2):
                pq = psum.tile([128, LT], f32, tag="ps512", name="ps512")
                k = 0
                for lo in range(2):
                    for cp in range(NP):
                        nc.tensor.matmul(
                            pq,
                            wq_hh[:, 2 * cp:2 * cp + 2, m * 128:(m + 1) * 128],
                            tf2[:, 2 * cp:2 * cp + 2, lo, :],
                            start=(k == 0), stop=(k == NMM - 1),
                            perf_mode=DRow)
                        k += 1
                nc.scalar.activation(
                    out=q_h[:, m, :], in_=pq, func=AF.Identity,
                    bias=bq_sb[:, h * 2 + m:h * 2 + m + 1], scale=RS)
            S.setdefault(i, {})["q"] = q_h

        def emit_K(i, c_outer=False):
            b, h = divmod(i, NH)
            if2 = TFI[b][1]
            wk_hh = W[i][1]
            k_h = atp.tile([128, 2, LI], f8, tag="kh", name="kh")
            passes = (0, 1)
            NP = NCI // 2
            NMM = len(passes) * NP

            def mm(pk, m, n, lo, cp, k):
                nc.tensor.matmul(
                    pk, wk_hh[:, 2 * cp:2 * cp + 2, m * 128:(m + 1) * 128],
                    if2[:, 2 * cp:2 * cp + 2, lo, n * 288:(n + 1) * 288],
                    start=(k == 0), stop=(k == NMM - 1), perf_mode=DRow)

            if c_outer:
                # chunk-pair-outer over 4 open psums, with a caller-provided
                # emission (pair-0's Q) interleaved after cp 1 so the PE has
                # work while the tail of the image stream arrives
                pks = {}
                for m in range(2):
                    for n in range(2):
                        pks[(m, n)] = psum.tile([128, 288], f32, tag="ps512",
                                                name="ps512")
                k = 0
                for cp in range(NP):
                    if cp == 2 and c_outer is not True:
                        c_outer()
                    for lo in passes:
                        for m in range(2):
                            for n in range(2):
                                mm(pks[(m, n)], m, n, lo, cp, k)
                        k += 1
                for m in range(2):
                    for n in range(2):
                        nc.scalar.activation(
                            out=k_h[:, m, n * 288:(n + 1) * 288],
                            in_=pks[(m, n)], func=AF.Identity,
                            bias=bk_sb[:, h * 2 + m:h * 2 + m + 1], scale=RS)
            else:
                for m in range(2):
                    for n in range(2):
                        pk = psum.tile([128, 288], f32, tag="ps512", name="ps512")
                        k = 0
                        for lo in passes:
                            for cp in range(NP):
                                mm(pk, m, n, lo, cp, k)
                                k += 1
                        nc.scalar.activation(
                            out=k_h[:, m, n * 288:(n + 1) * 288], in_=pk,
                            func=AF.Identity,
                            bias=bk_sb[:, h * 2 + m:h * 2 + m + 1], scale=RS)
            S[i]["k"] = k_h

        def emit_V(i, c_outer=False):
            b, h = divmod(i, NH)
            if2 = TFI[b][1]
            wv2 = W[i][2]
            v_h = atp.tile([128, 5, HD], bf16, tag="vh", name="vh")
            # image is the stationary side here: V = (ih+il)^T wvh + ih^T wvl
            passes = ((0, 0), (0, 1), (1, 0))  # (image lo, wv lo)
            NP = NCI // 2
            NMM = len(passes) * NP

            def mm(pv, t, pt, ilo, wlo, cp, k):
                nc.tensor.matmul(
                    pv[:pt],
                    if2[:, 2 * cp:2 * cp + 2, ilo, t * 128:t * 128 + pt],
                    wv2[:, 2 * cp:2 * cp + 2, wlo, :],
                    start=(k == 0), stop=(k == NMM - 1), perf_mode=DRow)

            if c_outer:
                pvs = {}
                for t, pt in ITC:
                    pvs[t] = psum.tile([128, HD], f32, tag="ps512", name="ps512")
                k = 0
                for cp in range(NP):
                    for ilo, wlo in passes:
                        for t, pt in ITC:
                            mm(pvs[t], t, pt, ilo, wlo, cp, k)
                        k += 1
                for t, pt in ITC:
                    nc.vector.tensor_scalar(
                        out=v_h[:pt, t, :], in0=pvs[t][:pt], scalar1=RS,
                        scalar2=None, op0=ALU.mult)
            else:
                for t, pt in ITC:
                    pv = psum.tile([128, HD], f32, tag="ps512", name="ps512")
                    k = 0
                    for ilo, wlo in passes:
                        for cp in range(NP):
                            mm(pv, t, pt, ilo, wlo, cp, k)
                            k += 1
                    nc.vector.tensor_scalar(
                        out=v_h[:pt, t, :], in0=pv[:pt], scalar1=RS,
                        scalar2=None, op0=ALU.mult)
            S[i]["v"] = v_h

        def emit_scores(i):
            q_h, k_h = S[i]["q"], S[i]["k"]
            e_f = atp.tile([128, 5, LT], bf16, tag="ef", name="ef")
            e8_f = atp.tile([128, 4, LT], f8, tag="e8f", name="e8f")
            for t, pt in ITC:
                ps_s = psum.tile([128, LT], f32, tag="ps512", name="ps512")
                nc.tensor.matmul(
                    ps_s[:pt], k_h[:, :, t * 128:t * 128 + pt], q_h[:, :, :],
                    start=True, stop=True, perf_mode=DRow)
                nc.scalar.activation(
                    out=e_f[:pt, t, :], in_=ps_s[:pt], func=AF.Exp, scale=ISCALE)
                # fp8 copy of e (for the DoubleRow denominator matmul; the 576
                # keys average the 2^-4 noise away) on the otherwise-idle
                # GPSIMD engine; px keeps the bf16 e. The 64-token tail chunk
                # sums in bf16 directly so pd never waits on the last copy.
                if t < 4:
                    nc.gpsimd.tensor_copy(out=e8_f[:pt, t, :],
                                          in_=e_f[:pt, t, :])
            S[i]["e"] = e_f
            S[i]["e8"] = e8_f

        def emit_scores_V(i):
            # interleave each scores chunk with a V chunk: the PE blasts the 5
            # scores matmuls in ~0.5us but the exp Acts drain their PSUM banks
            # ~570ns apiece, so back-to-back scores->V stalls on bank recycle
            q_h, k_h = S[i]["q"], S[i]["k"]
            if2 = TFI[divmod(i, NH)[0]][1]
            wv_hh = W[i][2]
            e_f = atp.tile([128, 5, LT], bf16, tag="ef", name="ef")
            e8_f = atp.tile([128, 4, LT], f8, tag="e8f", name="e8f")
            v_h = atp.tile([128, 5, HD], bf16, tag="vh", name="vh")
            passes = (0, 1)   # V = (ih+il)^T wvh (single-fp8 V weights)
            NP = NCI // 2
            NMM = len(passes) * NP
            for t, pt in ITC:
                ps_s = psum.tile([128, LT], f32, tag="ps512", name="ps512")
                nc.tensor.matmul(
                    ps_s[:pt], k_h[:, :, t * 128:t * 128 + pt], q_h[:, :, :],
                    start=True, stop=True, perf_mode=DRow)
                nc.scalar.activation(
                    out=e_f[:pt, t, :], in_=ps_s[:pt], func=AF.Exp, scale=ISCALE)
                if t < 4:
                    nc.gpsimd.tensor_copy(out=e8_f[:pt, t, :],
                                          in_=e_f[:pt, t, :])
                pv = psum.tile([128, HD], f32, tag="ps512", name="ps512")
                k = 0
                for ilo in passes:
                    for cp in range(NP):
                        nc.tensor.matmul(
                            pv[:pt],
                            if2[:, 2 * cp:2 * cp + 2, ilo,
                                t * 128:t * 128 + pt],
                            wv_hh[:, 2 * cp:2 * cp + 2, :],
                            start=(k == 0), stop=(k == NMM - 1),
                            perf_mode=DRow)
                        k += 1
                nc.vector.tensor_scalar(
                    out=v_h[:pt, t, :], in0=pv[:pt], scalar1=RS,
                    scalar2=None, op0=ALU.mult)
            S[i]["e"] = e_f
            S[i]["e8"] = e8_f
            S[i]["v"] = v_h

        def emit_pd(i):
            e8_f = S[i]["e8"]
            pd2 = psd.tile([1, LT], f32, tag="psd", name="psd")
            for tp in range(2):
                nc.tensor.matmul(pd2[0:1, :], ones8_pd[:, :, 0:1],
                                 e8_f[:, 2 * tp:2 * tp + 2, :],
                                 start=(tp == 0), stop=False, perf_mode=DRow)
            nc.tensor.matmul(pd2[0:1, :], ones_col_b[:64],
                             S[i]["e"][:64, 4, :], start=False, stop=True)
            recip = atp.tile([1, LT], bf16, tag="recip", name="recip")
            with nc.allow_low_precision(reason="softmax recip feeds bf16 matmul"):
                nc.vector.reciprocal(out=recip, in_=pd2[0:1, :])
            S[i]["recip"] = recip

        def emit_pbc(i):
            # broadcast the softmax reciprocal across partitions on the
            # (mostly idle) GPSIMD instead of a PE ones-matmul + Act copy
            bcast = atp.tile([128, LT], bf16, tag="bcast", name="bcast")
            nc.gpsimd.partition_broadcast(bcast, S[i]["recip"])
            S[i]["bcast"] = bcast

        def emit_px(i):
            emit_px_mms(i)
            emit_px_mult(i)

        def emit_px_mms(i):
            # the attention-value matmuls depend only on v/e, not on the
            # softmax reciprocal: emitting them before pd gives the GPSIMD
            # e8 copies (pd's DoubleRow operands) ~2us more cover
            v_h, e_f = S[i]["v"], S[i]["e"]
            pxs = []
            for m in range(2):
                px = psum.tile([128, LT], f32, tag="ps512", name="ps512")
                for t, pt in ITC:
                    nc.tensor.matmul(
                        px, v_h[:pt, t, m * 128:(m + 1) * 128], e_f[:pt, t, :],
                        start=(t == 0), stop=(t == 4))
                pxs.append(px)
            S[i]["pxs"] = pxs

        def emit_px_mult(i):
            b, h = divmod(i, NH)
            bcast = S[i]["bcast"]
            for m in range(2):
                nc.vector.tensor_mul(out=r(XF[b][h * 2 + m]),
                                     in0=S[i]["pxs"][m], in1=bcast)
            S[i] = None  # release references

        def emit_rev(b, split_first=None, fuse_ff1=False):
            ofs = [ofp.tile([128, LT], f32, tag=f"of{m}", name=f"of{m}")
                   for m in range(NCD)]
            ofb = [ofp.tile([128, LT], bf16, tag=f"ofb{m}", name=f"ofb{m}")
                   for m in range(NCD)]
            NC16 = H // 128
            # fused FF1: emit each FF1 matmul lagged two chunks behind its
            # ofb writeback so the PE never waits on the Act/DVE queue; this
            # pulls FF1 inside rev's span and shortens the serial tail chain.
            # ph lives in the psd pool (same 2KB bank footprint as pstA/B).
            ph = psd.tile([128, LT], f32, tag="psd", name="psd") \
                if fuse_ff1 else None

            def ff1mm(m):
                nc.tensor.matmul(ph, w1_sb[:, m, :], ofb[m],
                                 start=(m == 0), stop=(m == NCD - 1))

            def mm(po, m, c, start, stop):
                nc.tensor.matmul(po, r(wr_sb[:, c, m * 128:(m + 1) * 128]),
                                 r(XF[b][c]), start=start, stop=stop)

            def writeback(m, po):
                if m < NCD - 1:
                    nc.scalar.activation(
                        out=r(ofs[m]), in_=po, func=AF.Identity,
                        bias=breff_sb[:, m:m + 1], scale=1.0)
                    nc.vector.tensor_scalar(
                        out=ofb[m], in0=po, scalar1=breff_sb[:, m:m + 1],
                        scalar2=None, op0=ALU.add)
                else:
                    # last chunk: ofb gates FF1's final matmul - put it on the
                    # idle Act queue, ofs on DVE
                    nc.scalar.activation(
                        out=ofb[m], in_=po, func=AF.Identity,
                        bias=breff_sb[:, m:m + 1], scale=1.0)
                    nc.vector.tensor_scalar(
                        out=r(ofs[m]), in0=po, scalar1=breff_sb[:, m:m + 1],
                        scalar2=None, op0=ALU.add)

            if split_first is not None:
                # overlap the pair-15 softmax tail and its xf writes with
                # partial accumulations of the first two output chunks
                po0 = psum.tile([128, LT], f32, tag="ps512", name="ps512")
                for c in range(NC16 - 2):
                    mm(po0, 0, c, c == 0, False)
                split_first()
                po1 = psum.tile([128, LT], f32, tag="ps512", name="ps512")
                for c in range(NC16 - 2):
                    mm(po1, 1, c, c == 0, False)
                for c in (NC16 - 2, NC16 - 1):
                    mm(po0, 0, c, False, c == NC16 - 1)
                writeback(0, po0)
                for c in (NC16 - 2, NC16 - 1):
                    mm(po1, 1, c, False, c == NC16 - 1)
                writeback(1, po1)
                first = 2
            else:
                first = 0
            for m in range(first, NCD):
                po = psum.tile([128, LT], f32, tag="ps512", name="ps512")
                for c in range(NC16):
                    mm(po, m, c, c == 0, c == NC16 - 1)
                writeback(m, po)
                if fuse_ff1 and m >= 2:
                    ff1mm(m - 2)
            if fuse_ff1:
                ff1mm(NCD - 2)
                ff1mm(NCD - 1)
            OF[b] = (ofs, ofb)
            return ph

        def emit_ff_stats(b, mid=None, ph=None):
            ofs, ofb = OF[b]
            i32 = mybir.dt.int32
            if ph is None:
                ph = psum.tile([128, LT], f32, tag="ps512", name="ps512")
                for c in range(NCD):
                    nc.tensor.matmul(ph, w1_sb[:, c, :], ofb[c],
                                     start=(c == 0), stop=(c == NCD - 1))
            h_sb = p5p.tile([128, LT], bf16, tag="hsb", name="hsb")
            nc.scalar.activation(out=h_sb, in_=ph, func=AF.Relu, bias=b1_sb,
                                 scale=1.0)
            if mid is not None:
                mid()   # PE work to cover the FF1 -> relu -> FF2 round trip
            pstA = psd.tile([16, LT], f32, tag="psd", name="psd")
            pstB = psd.tile([16, LT], f32, tag="psd", name="psd")

            def pf_mm(m):
                pf = psum.tile([128, LT], f32, tag="ps512", name="ps512")
                if m % 2 == 0:
                    # DVE path: fused ofs[m] = (pf + b2[m]) + ofs[m]
                    nc.tensor.matmul(pf, w2_sb[:, m * 128:(m + 1) * 128], h_sb)
                    nc.vector.scalar_tensor_tensor(
                        out=r(ofs[m]), in0=pf, scalar=b2_sb[:, m:m + 1],
                        in1=ofs[m], op0=ALU.add, op1=ALU.add)
                    nc.scalar.activation(out=ofb[m], in_=ofs[m],
                                         func=AF.Square, scale=1.0)
                else:
                    # PE path: residual via identity-accumulate, Act writeback
                    nc.tensor.matmul(pf, w2_sb[:, m * 128:(m + 1) * 128], h_sb,
                                     start=True, stop=False)
                    nc.tensor.matmul(pf, r(ident_r), r(ofs[m]), start=False,
                                     stop=True)
                    nc.scalar.activation(out=r(ofs[m]), in_=pf,
                                         func=AF.Identity,
                                         bias=b2_sb[:, m:m + 1], scale=1.0)
                    nc.vector.tensor_mul(out=ofb[m], in0=ofs[m], in1=ofs[m])

            def stA(m):
                nc.tensor.matmul(pstA, r(ones16_f), r(ofs[m]),
                                 start=(m == 0), stop=(m == NCD - 1))

            def stB(m):
                nc.tensor.matmul(pstB, ones16_b, ofb[m],
                                 start=(m == 0), stop=(m == NCD - 1))

            # interleave stats accumulation behind the pf matmuls so PE never
            # waits on the stt/sq chains
            pf_mm(0)
            pf_mm(1)
            stA(0)
            pf_mm(2)
            stA(1)
            stB(0)
            pf_mm(3)
            stA(2)
            stB(1)
            pf_mm(4)
            stA(3)
            stB(2)
            pf_mm(5)
            stA(4)
            stB(3)
            stA(5)
            stB(4)
            stB(5)

            srowA = ofp.tile([16, LT], f32, tag="srowA", name="srowA")
            srowB = ofp.tile([16, LT], f32, tag="srowB", name="srowB")
            nc.scalar.activation(out=r(srowA), in_=pstA, func=AF.Copy)
            nc.scalar.activation(out=r(srowB), in_=pstB, func=AF.Copy)
            mq4 = p5p.tile([128, NT, 32], f32, tag="mq4", name="mq4")
            ptc = psum.tile([128, NT, 32], f32, tag="ps512", name="ps512")
            for t in range(NT):
                nc.tensor.transpose(r(ptc[:, t, 0:16]),
                                    r(srowA[:, t * 128:(t + 1) * 128]),
                                    r(ident_r[:16, :16]))
                nc.tensor.transpose(r(ptc[:, t, 16:32]),
                                    r(srowB[:, t * 128:(t + 1) * 128]),
                                    r(ident_r[:16, :16]))
            nc.vector.tensor_scalar(out=mq4, in0=ptc, scalar1=1.0 / DT,
                                    scalar2=None, op0=ALU.mult)
            mu4 = mq4[:, :, 0]
            q4 = mq4[:, :, 16]
            musq = p5p.tile([128, NT], f32, tag="musq4", name="musq4")
            nc.vector.tensor_mul(out=musq, in0=mu4, in1=mu4)
            v4 = p5p.tile([128, NT], f32, tag="v4", name="v4")
            nc.vector.tensor_sub(out=v4, in0=q4, in1=musq)
            # rstd = rsqrt(v4): quake bit-trick seed + Newton steps, DVE only
            y4 = p5p.tile([128, NT], f32, tag="y4", name="y4")
            sh4 = p5p.tile([128, NT], f32, tag="sh4", name="sh4")
            nc.vector.tensor_scalar(
                out=sh4.bitcast(i32), in0=v4.bitcast(i32), scalar1=1,
                scalar2=None, op0=ALU.logical_shift_right)
            nc.vector.tensor_sub(out=y4.bitcast(i32), in0=magic4.bitcast(i32),
                                 in1=sh4.bitcast(i32))
            t14 = p5p.tile([128, NT], f32, tag="t14", name="t14")
            for _ in range(1):
                nc.vector.tensor_mul(out=t14, in0=y4, in1=y4)
                nc.vector.tensor_mul(out=t14, in0=t14, in1=v4)
                nc.vector.tensor_scalar(out=t14, in0=t14, scalar1=-0.5,
                                        scalar2=1.5, op0=ALU.mult, op1=ALU.add)
                nc.vector.tensor_mul(out=y4, in0=y4, in1=t14)
            nmr4 = p5p.tile([128, NT], f32, tag="nmr4", name="nmr4")
            nc.vector.scalar_tensor_tensor(
                out=nmr4, in0=mu4, scalar=-1.0, op0=ALU.mult, in1=y4,
                op1=ALU.mult)
            stats = [(y4[:, t:t + 1], nmr4[:, t:t + 1]) for t in range(NT)]
            OF[b] = (ofs, stats)

        def emit_ln_chunk(b, t):
            ofs, stats = OF[b]
            rstd, nmr = stats[t]
            # batch 1 (the tail) gets a distinct y tile per chunk so the four
            # normalize->store chains pipeline instead of serializing on tile
            # reuse; batch 0's chunks are spread across pairs and can share
            ytag = f"y{t % 3}" if b == 1 else "y0"
            ypool = ofp if b == 1 else p5p   # ofp is bufs=1: no double alloc
            y = ypool.tile([128, DT], f32, tag=ytag, name=ytag)
            for half in range(2):
                ptr_ = psum.tile([128, 384], f32, tag="ps512", name="ps512")
                for j in range(3):
                    c = half * 3 + j
                    nc.tensor.transpose(
                        r(ptr_[:, j * 128:(j + 1) * 128]),
                        r(ofs[c][:, t * 128:(t + 1) * 128]), r(ident_r))
                dst = y[:, half * 384:(half + 1) * 384]
                if half == 0:
                    nc.scalar.activation(out=dst, in_=ptr_, func=AF.Identity,
                                         bias=nmr, scale=rstd)
                else:
                    nc.vector.tensor_scalar(
                        out=dst, in0=ptr_, scalar1=rstd, scalar2=nmr,
                        op0=ALU.mult, op1=ALU.add)
                if apply_gamma:
                    nc.vector.tensor_mul(out=dst, in0=dst,
                                         in1=gam_sb[:, half * 384:(half + 1) * 384])
                    nc.vector.tensor_add(out=dst, in0=dst,
                                         in1=bet_sb[:, half * 384:(half + 1) * 384])
            # one whole-chunk store per (b, t) on the SP queue: fewest
            # HWDGE slots and DMA-sem edges
            nc.sync.dma_start(out=out[b, t * 128:(t + 1) * 128, :], in_=y)

        def load_text(b, eng=None):
            tf2 = tfp.tile([128, NCD, 2, LT], f8, tag="textf2", name="textf2")
            (eng or nc.sync).dma_start(out=tf2, in_=t2p[b])
            return tf2

        def alloc_image():
            return tfp.tile([128, NCI, 2, LI], f8, tag="imgf2", name="imgf2")

        def load_image_part(if2, b, c0, c1, eng=None):
            (eng or nc.sync).dma_start(
                out=if2[:, c0:c1], in_=i2p[b, :, c0:c1])

        def load_image(b, eng=None):
            if2 = alloc_image()
            load_image_part(if2, b, 0, 8, eng)
            return if2

        # ---------------- emission schedule ----------------
        # startup: the HWDGE serializes ~625ns per DMA instruction, each DMA
        # dependency edge pays a 900ns sem-prop, and the bus runs ~330GB/s.
        # Order the queue by pair-0's fine-grained consumption and interleave
        # K and Q chunk-group matmuls with the piece arrivals.
        wk_h0h = hwp.tile([128, NCI, HD], f8, tag="wkhh", name="wkhh")
        nc.sync.dma_start(out=wk_h0h, in_=wkph[:, 0])
        if0 = alloc_image()
        load_image_part(if0, 0, 0, 2)   # image chunks 0-1 (h+l)
        wq_h0h = hwp.tile([128, NCD, HD], f8, tag="wqhh", name="wqhh")
        nc.sync.dma_start(out=wq_h0h, in_=wqph[:, 0])
        tfs0 = tfp.tile([128, NCD, 2, LT], f8, tag="textf2", name="textf2")
        nc.sync.dma_start(out=tfs0[:, 0:4], in_=t2p[0, :, 0:4])
        load_image_part(if0, 0, 2, 4)
        # biases mid-queue: tiny transfers, but each costs a 625ns shared
        # HWDGE slot; at the queue head they delay the first K matmul
        nc.scalar.dma_start(out=bk_sb, in_=bkp)
        nc.scalar.dma_start(out=bq_sb, in_=bqp)
        nc.sync.dma_start(out=tfs0[:, 4:6], in_=t2p[0, :, 4:6])
        load_image_part(if0, 0, 4, 8)
        wv_h0h = hwp.tile([128, NCI, HD], f8, tag="wvhh", name="wvhh")
        nc.sync.dma_start(out=wv_h0h, in_=wvph[:, 0])
        TFI[0] = (tfs0, if0)
        W[0] = (wq_h0h, wk_h0h, wv_h0h)
        alloc_xf(0)

        # ---- pair 0: K/Q chunk-groups emitted in DMA-arrival order ----
        S[0] = {}
        pks0 = {}
        for m in range(2):
            for n in range(2):
                pks0[(m, n)] = psum.tile([128, 288], f32, tag="ps512",
                                         name="ps512")

        def k0_cp(cp, first, last):
            for lo in range(2):
                for m in range(2):
                    for n in range(2):
                        nc.tensor.matmul(
                            pks0[(m, n)],
                            wk_h0h[:, 2 * cp:2 * cp + 2, m * 128:(m + 1) * 128],
                            if0[:, 2 * cp:2 * cp + 2, lo,
                                n * 288:(n + 1) * 288],
                            start=(first and lo == 0), stop=(last and lo == 1),
                            perf_mode=DRow)

        def q0_cp(cp, first, last):
            for m in range(2):
                for lo in range(2):
                    nc.tensor.matmul(
                        pqs0[m],
                        wq_h0h[:, 2 * cp:2 * cp + 2, m * 128:(m + 1) * 128],
                        tfs0[:, 2 * cp:2 * cp + 2, lo, :],
                        start=(first and lo == 0), stop=(last and lo == 1),
                        perf_mode=DRow)

        k0_cp(0, True, False)
        pqs0 = [psum.tile([128, LT], f32, tag="ps512", name="ps512")
                for _ in range(2)]
        q0_cp(0, True, False)
        k0_cp(1, False, False)
        q0_cp(1, False, False)
        q0_cp(2, False, True)
        q_h0 = atp.tile([128, 2, LT], f8, tag="qh", name="qh")
        for m in range(2):
            nc.scalar.activation(out=q_h0[:, m, :], in_=pqs0[m],
                                 func=AF.Identity, bias=bq_sb[:, m:m + 1],
                                 scale=RS)
        S[0]["q"] = q_h0
        k0_cp(2, False, False)
        k0_cp(3, False, True)
        k_h0 = atp.tile([128, 2, LI], f8, tag="kh", name="kh")
        for m in range(2):
            for n in range(2):
                nc.scalar.activation(
                    out=k_h0[:, m, n * 288:(n + 1) * 288], in_=pks0[(m, n)],
                    func=AF.Identity, bias=bk_sb[:, m:m + 1], scale=RS)
        S[0]["k"] = k_h0
        emit_scores_V(0)
        load_weights(1)
        load_weights(2)

        prev = 0
        for i in range(1, NPAIR - 1):
            if i + 2 <= NPAIR - 1:
                load_weights(i + 2)   # 2-pair prefetch distance (hwp bufs=3)
            if i == 2:
                load_consts_late()
            if i == 5:
                TFI1_T = load_text(1)
            if i == 6:
                IF1 = alloc_image()
                load_image_part(IF1, 1, 0, 4)
            if i == 7:
                load_image_part(IF1, 1, 4, 8)
                TFI[1] = (TFI1_T, IF1)
            if 1 <= i <= 8:
                load_wr_chunk(i - 1)
            emit_Q(i)
            if i == 9:
                # fill the FF1->relu->FF2 round trip of batch 0 with
                # pair-9's K matmuls
                emit_ff_stats(0, mid=lambda: emit_K(i))
            else:
                emit_K(i)
            # px matmuls first, then pd: pd's DoubleRow operands are the
            # GPSIMD e8 copies of the previous pair (~3.4us after the exps)
            emit_px_mms(prev)
            emit_pd(prev)
            emit_pbc(prev)
            emit_px_mult(prev)
            emit_scores_V(i)
            if i == 8:
                emit_rev(0)
                alloc_xf(1)
            elif 10 <= i <= 13:
                emit_ln_chunk(0, i - 10)
            prev = i
        # ---- block 15: scores before V so exp(15) finishes during V;
        # reversion's first group splits around the pair-15 softmax tail ----
        i = NPAIR - 1
        emit_Q(i)
        emit_K(i)
        emit_px_mms(prev)
        emit_pd(prev)
        emit_pbc(prev)
        emit_px_mult(prev)
        emit_scores_V(i)
        emit_pd(i)

        def _tail15():
            emit_pbc(i)
            emit_px(i)

        emit_rev(1, split_first=_tail15)
        emit_ff_stats(1)
        for t in range(NT):
            emit_ln_chunk(1, t)

    nc.compile()
    _BUILD_CACHE[key] = nc
    return nc


def _prep_in_maps(inputs):
    import ml_dtypes

    F8 = ml_dtypes.float8_e4m3

    def bf(x):
        return np.ascontiguousarray(np.asarray(x, dtype=np.float32).astype(
            ml_dtypes.bfloat16))

    def f32c(x):
        return np.ascontiguousarray(np.asarray(x, dtype=np.float32))

    def split8(x):
        x = np.asarray(x, np.float32)
        xh = x.astype(F8)
        xl = (x - xh.astype(np.float32)).astype(F8)
        return xh, xl

    def featmajor(x, L, D):
        # [B_TOTAL, L, D] -> [B_TOTAL, 128, D//128, L]
        return np.ascontiguousarray(
            np.asarray(x).reshape(-1, L, D // 128, 128).transpose(0, 3, 2, 1))

    def headmajor(w, din):
        # [din, H] -> [128, NH, din//128, HD] (per-partition contiguous runs)
        return np.ascontiguousarray(
            np.asarray(w).reshape(din // 128, 128, NH, HD).transpose(1, 2, 0, 3))

    th, tl = split8(inputs["text"])
    ih, il = split8(inputs["image"])
    t2 = np.ascontiguousarray(np.stack(
        (featmajor(th, LT, DT), featmajor(tl, LT, DT)), axis=3))
    i2 = np.ascontiguousarray(np.stack(
        (featmajor(ih, LI, DI), featmajor(il, LI, DI)), axis=3))
    wqh = (np.asarray(inputs["wq"], np.float32) * WS).astype(F8)
    wkh = (np.asarray(inputs["wk"], np.float32) * WS).astype(F8)
    wvh = (np.asarray(inputs["wv"], np.float32) * WS).astype(F8)
    wr = np.asarray(inputs["wr"], dtype=np.float64)
    bv = np.asarray(inputs["bv"], dtype=np.float64)
    br = np.asarray(inputs["br"], dtype=np.float64)
    breff = (br + bv @ wr).astype(np.float32)
    w1 = bf(inputs["w1"])
    gamma = f32c(inputs["gamma"])
    beta = f32c(inputs["beta"])

    shared = {
        "wqph": headmajor(wqh, DT),
        "wkph": headmajor(wkh, DI),
        "wvph": headmajor(wvh, DI),
        "wrp": np.ascontiguousarray(
            f32c(inputs["wr"]).reshape(H // 128, 128, DT).transpose(1, 0, 2)),
        "w1p": np.ascontiguousarray(
            w1.reshape(NCD, 128, FF).transpose(1, 0, 2)),
        "w2": bf(inputs["w2"]),
        "bqp": np.ascontiguousarray(
            f32c(inputs["bq"]).reshape(H // 128, 128).T),
        "bkp": np.ascontiguousarray(
            f32c(inputs["bk"]).reshape(H // 128, 128).T),
        "b1p": np.ascontiguousarray(f32c(inputs["b1"]).reshape(128, 1)),
        "b2p": np.ascontiguousarray(f32c(inputs["b2"]).reshape(NCD, 128).T),
        "breffp": np.ascontiguousarray(breff.reshape(NCD, 128).T),
        "gamp": np.ascontiguousarray(np.broadcast_to(gamma, (128, DT))),
        "betp": np.ascontiguousarray(np.broadcast_to(beta, (128, DT))),
    }
    in_maps = []
    for c in range(N_CORES):
        m = dict(shared)
        m["t2p"] = t2[c * B:(c + 1) * B]
        m["i2p"] = i2[c * B:(c + 1) * B]
        in_maps.append(m)
    return in_maps


def _needs_gamma(inputs):
    g = np.asarray(inputs["gamma"], dtype=np.float32)
    b = np.asarray(inputs["beta"], dtype=np.float32)
    return not (np.all(g == 1.0) and np.all(b == 0.0))


def kernel(**inputs) -> np.ndarray:
    _ensure_import_path()
    from concourse.bass_utils import run_bass_kernel_spmd

    nc = build_module(apply_gamma=_needs_gamma(inputs))
    in_maps = _prep_in_maps(inputs)
    res = run_bass_kernel_spmd(nc, in_maps, core_ids=list(range(N_CORES)))
    return np.concatenate([res.results[c]["out"] for c in range(N_CORES)], axis=0)


# revision 70
# speedup vs baseline: 1.0832x; 1.0832x over previous
"""Trainium2 Bass kernel for nn_AttentionLayer (cross-attention + FF + LayerNorm).

Strategy (data-parallel over batch, 2 batch elements per core, no collectives):
  - fp8e4m3 DoubleRow matmuls (256-deep contraction, 0.5 cyc/row in the
    TimelineSim cost model, i.e. 4x cheaper than bf16 per unit contraction)
    for the Q/K/V projections and the scores matmul, with error-compensated
    "split fp8": text/image ship as fp8-high + fp8-residual pairs
    (interleaved per chunk in one tensor: [.., chunk, 2, len]); weights are
    scaled x32 host-side (uniform(+-1/sqrt(fanin)) weights sit in fp8's
    subnormal range unscaled), un-scaled for free via the Act writeback
    scale=1/32. Q/K/V all run 2 passes W(Xh+Xl) with single-fp8 weights
    (the weight-residual passes were dropped one by one against the
    measured error budget). Q/K are stored fp8 so scores is one DoubleRow
    matmul per token chunk. HW end-to-end rel err 1.75e-2 vs the 2e-2
    gate (measured on the real cores; deterministic).
  - The softmax denominator uses a DoubleRow ones-matmul over an fp8 copy
    of e made on the (otherwise idle) GPSIMD; the 64-token tail chunk sums
    the bf16 e directly so pd never waits on the last copy. The reciprocal
    is broadcast across partitions by GPSIMD partition_broadcast (replaces
    a PE ones-matmul + Act copy). e / V / px stay bf16, reversion (wr, xf)
    and the residual/LN path stay f32r, so the dominant error terms stay
    high precision.
  - Inputs loaded FEATURE-major as pre-transposed fp8 from the host (plain
    contiguous DMAs, no XBAR transpose; same bytes as bf16). Few, large
    DMAs: the HWDGE serializes ~625ns per DMA instruction and each DMA
    dependency edge pays a ~900ns sem-prop, so instruction count matters
    more than piece size. Startup orders the queue by pair-0's consumption
    and interleaves K/Q chunk-group matmuls with the piece arrivals; wr
    (6.3MB) is spread over pairs 1-8 so per-pair weight loads never queue
    behind it; weights prefetch 2 pairs ahead.
  - Per pair: scores and V chunks interleave (the exp Acts drain score
    PSUM banks slower than the PE can issue, V chunks cover the recycle);
    the softmax tail of pair i-1 (pd, recip, broadcast, V^T@E) is emitted
    inside pair i's projections so PE never stalls on Act/DVE round trips.
  - Softmax un-normalized in [key, query] layout; normalization folded
    into the PSUM->SBUF move of x (TT mult with the broadcast reciprocal).
  - FF2 residual+bias: even chunks fused into one scalar_tensor_tensor
    (DVE), odd chunks accumulate the residual on the PE via an identity
    matmul with Act writeback; batch 0's FF1->relu->FF2 round trip is
    covered by pair-9's K matmuls; LayerNorm stats as ones-matmuls
    interleaved behind FF2, transposed to per-token columns via
    16-row-replicated PE transposes; rstd via quake bit-trick + one Newton
    step (DVE only); normalize fused into the PSUM->SBUF copies after the
    transpose back to token-major (Act/DVE halves); one whole-chunk output
    DMA per (batch, token-chunk), batch-1 tail chunks on distinct y tiles
    so the final stores pipeline.
  - V bias folded into the reversion bias host-side: breff = br + bv @ wr.
  - gamma/beta are applied only when not identity (build-time
    specialization keyed on the actual input values).

TimelineSim: 215.3 us (prior session's bf16 kernel: 344.4 us; original
baseline: 515 us). PE busy ~192 us (~89%). HW rel err 1.753e-2 (PASS).
"""

import os
import sys

import numpy as np

# ---- problem constants (hardcoded per contract) ----
B_TOTAL = 16
N_CORES = 8
B = B_TOTAL // N_CORES  # per-core batch
LT, DT = 512, 768       # text tokens / dim
LI, DI = 576, 1024      # image tokens / dim
H, NH, HD = 2048, 8, 256
FF = 128
ISCALE = 1.0 / 16.0     # 1/sqrt(HD)
NPAIR = B * NH          # 16 (batch, head) pairs per core
ITC = [(t, 128 if t < 4 else LI - 512) for t in range(5)]  # image tok chunks
NCD = DT // 128         # 6
NCI = DI // 128         # 8
NT = LT // 128          # 4
WS = 32.0               # host-side fp8 weight pre-scale
RS = 1.0 / WS

_BUILD_CACHE: dict = {}


def _ensure_import_path():
    try:
        import concourse  # noqa: F401
    except ModuleNotFoundError:
        for p in ("/opt/trn_rl_repo", "/root/.axon_site/_ro/trn_rl_repo"):
            if os.path.isdir(p) and p not in sys.path:
                sys.path.insert(0, p)


def build_module(apply_gamma: bool = False):
    key = ("v4", apply_gamma)
    if key in _BUILD_CACHE:
        return _BUILD_CACHE[key]
    _ensure_import_path()
    from contextlib import ExitStack

    import concourse.bacc as bacc
    import concourse.bass as bass  # noqa: F401
    import concourse.mybir as mybir
    import concourse.tile as tile
    from concourse.masks import make_identity

    f32 = mybir.dt.float32
    f32r = mybir.dt.float32r
    bf16 = mybir.dt.bfloat16
    f8 = mybir.dt.float8e4
    AF = mybir.ActivationFunctionType
    ALU = mybir.AluOpType
    DRow = mybir.MatmulPerfMode.DoubleRow

    def r(ap):
        return ap.bitcast(f32r)

    nc = bacc.Bacc("TRN2", target_bir_lowering=False, debug=False, num_devices=N_CORES)

    # text/image ship as [.., chunk, 2, len] with the fp8 high/low split
    # parts interleaved per chunk: one big DMA per tensor (fewer HWDGE slots
    # and 900ns DMA-sem edges), and the DoubleRow slices address either part
    t2p = nc.dram_tensor("t2p", [B, 128, NCD, 2, LT], f8,
                         kind="ExternalInput").ap()
    i2p = nc.dram_tensor("i2p", [B, 128, NCI, 2, LI], f8,
                         kind="ExternalInput").ap()
    wqph = nc.dram_tensor("wqph", [128, NH, NCD, HD], f8,
                          kind="ExternalInput").ap()
    wkph = nc.dram_tensor("wkph", [128, NH, NCI, HD], f8,
                          kind="ExternalInput").ap()
    wvph = nc.dram_tensor("wvph", [128, NH, NCI, HD], f8,
                          kind="ExternalInput").ap()
    wrp = nc.dram_tensor("wrp", [128, H // 128, DT], f32,
                         kind="ExternalInput").ap()
    w1p = nc.dram_tensor("w1p", [128, NCD, FF], bf16, kind="ExternalInput").ap()
    w2 = nc.dram_tensor("w2", [FF, DT], bf16, kind="ExternalInput").ap()
    bqp = nc.dram_tensor("bqp", [128, H // 128], f32, kind="ExternalInput").ap()
    bkp = nc.dram_tensor("bkp", [128, H // 128], f32, kind="ExternalInput").ap()
    b1p = nc.dram_tensor("b1p", [128, 1], f32, kind="ExternalInput").ap()
    b2p = nc.dram_tensor("b2p", [128, NCD], f32, kind="ExternalInput").ap()
    breffp = nc.dram_tensor("breffp", [128, NCD], f32,
                            kind="ExternalInput").ap()
    gamp = nc.dram_tensor("gamp", [128, DT], f32, kind="ExternalInput").ap()
    betp = nc.dram_tensor("betp", [128, DT], f32, kind="ExternalInput").ap()
    out = nc.dram_tensor("out", [B, LT, DT], f32, kind="ExternalOutput").ap()

    with tile.TileContext(nc) as tc, ExitStack() as ctx:
        const = ctx.enter_context(tc.tile_pool(name="const", bufs=1))
        ident = const.tile([128, 128], f32)
        make_identity(nc, ident)
        # the copy performs the required fp32 -> fp32r rounding for the
        # f32r matmul consumers (BIR verifier rejects unrounded producers)
        ident_r = const.tile([128, 128], f32)
        nc.vector.tensor_copy(out=r(ident_r), in_=ident)
        ones_col_b = const.tile([128, 1], bf16)
        nc.vector.memset(ones_col_b, 1.0)
        # fp8 ones for the DoubleRow softmax-denominator matmul; middle-dim
        # stride kept at 16B for the DR weight AP alignment requirement
        ones8_pd = const.tile([128, 2, 16], f8)
        nc.vector.memset(ones8_pd, 1.0)
        ones_row_b = const.tile([1, 128], bf16)
        nc.vector.memset(ones_row_b, 1.0)
        ones_tmp = const.tile([128, 16], f32)
        nc.vector.memset(ones_tmp, 1.0)
        ones16_f = const.tile([128, 16], f32)
        nc.vector.tensor_copy(out=r(ones16_f), in_=ones_tmp)
        ones16_b = const.tile([128, 16], bf16)
        nc.vector.memset(ones16_b, 1.0)
        # float with bit pattern 0x5f3759df (quake rsqrt magic)
        magic4 = const.tile([128, NT], f32)
        nc.vector.memset(magic4, float(np.uint32(0x5F3759DF).view(np.float32)))
        # dummy activation: forces the act-table load off the critical path
        warm = const.tile([1, 1], f32)
        nc.scalar.activation(out=warm, in_=magic4[0:1, 0:1], func=AF.Exp,
                             scale=0.0)

        bq_sb = const.tile([128, H // 128], f32)
        bk_sb = const.tile([128, H // 128], f32)
        b1_sb = const.tile([128, 1], f32)
        b2_sb = const.tile([128, NCD], f32)
        breff_sb = const.tile([128, NCD], f32)
        w1_sb = const.tile([128, NCD, FF], bf16)
        w2_sb = const.tile([128, DT], bf16)
        wr_sb = const.tile([128, H // 128, DT], f32)  # loaded in 4 chunks mid-flight
        gam_sb = bet_sb = None
        if apply_gamma:
            gam_sb = const.tile([128, DT], f32)
            bet_sb = const.tile([128, DT], f32)

        def load_consts_late():
            nc.sync.dma_start(out=b1_sb, in_=b1p)
            nc.sync.dma_start(out=b2_sb, in_=b2p)
            nc.sync.dma_start(out=breff_sb, in_=breffp)
            nc.sync.dma_start(out=w1_sb, in_=w1p)
            nc.sync.dma_start(out=w2_sb, in_=w2)
            if apply_gamma:
                nc.sync.dma_start(out=gam_sb, in_=gamp)
                nc.sync.dma_start(out=bet_sb, in_=betp)

        psum = ctx.enter_context(tc.tile_pool(name="psum", bufs=6, space="PSUM"))
        psd = ctx.enter_context(tc.tile_pool(name="psd", bufs=2, space="PSUM"))
        tfp = ctx.enter_context(tc.tile_pool(name="tfp", bufs=2))
        hwp = ctx.enter_context(tc.tile_pool(name="hwp", bufs=3))
        atp = ctx.enter_context(tc.tile_pool(name="atp", bufs=2))
        xfp = ctx.enter_context(tc.tile_pool(name="xfp", bufs=1))
        ofp = ctx.enter_context(tc.tile_pool(name="ofp", bufs=1))
        p5p = ctx.enter_context(tc.tile_pool(name="p5p", bufs=2))

        TFI: dict = {}
        W: dict = {}
        S: dict = {}
        XF: dict = {}
        OF: dict = {}
        SQ: dict = {}

        def load_weights(i, eng=None):
            eng = eng or nc.sync
            h = i % NH
            wq_hh = hwp.tile([128, NCD, HD], f8, tag="wqhh", name="wqhh")
            eng.dma_start(out=wq_hh, in_=wqph[:, h])
            wk_hh = hwp.tile([128, NCI, HD], f8, tag="wkhh", name="wkhh")
            eng.dma_start(out=wk_hh, in_=wkph[:, h])
            wv_hh = hwp.tile([128, NCI, HD], f8, tag="wvhh", name="wvhh")
            eng.dma_start(out=wv_hh, in_=wvph[:, h])
            W[i] = (wq_hh, wk_hh, wv_hh)

        def load_wr_chunk(cc):
            # wr is 6.3MB (~19us of bus): spread it thinly (8 chunks over
            # pairs 1-8) so the SP queue's per-pair weight loads never sit
            # behind a multi-us transfer
            nc.sync.dma_start(
                out=r(wr_sb[:, cc * 2:(cc + 1) * 2, :]),
                in_=r(wrp[:, cc * 2:(cc + 1) * 2, :]))

        def alloc_xf(b):
            XF[b] = [xfp.tile([128, LT], f32, tag=f"xf{c}", name=f"xf{c}")
                     for c in range(H // 128)]

        def emit_Q(i):
            b, h = divmod(i, NH)
            tf2 = TFI[b][0]
            wq_hh = W[i][0]
            q_h = atp.tile([128, 2, LT], f8, tag="qh", name="qh")
            NP = NCD // 2
            NMM = 2 * NP
            for m in range(2):
                pq = psum.tile([128, LT], f32, tag="ps512", name="ps512")
                k = 0
                for lo in range(2):
                    for cp in range(NP):
                        nc.tensor.matmul(
                            pq,
                            wq_hh[:, 2 * cp:2 * cp + 2, m * 128:(m + 1) * 128],
                            tf2[:, 2 * cp:2 * cp + 2, lo, :],
                            start=(k == 0), stop=(k == NMM - 1),
                            perf_mode=DRow)
                        k += 1
                nc.scalar.activation(
                    out=q_h[:, m, :], in_=pq, func=AF.Identity,
                    bias=bq_sb[:, h * 2 + m:h * 2 + m + 1], scale=RS)
            S.setdefault(i, {})["q"] = q_h

        def emit_K(i, c_outer=False):
            b, h = divmod(i, NH)
            if2 = TFI[b][1]
            wk_hh = W[i][1]
            k_h = atp.tile([128, 2, LI], f8, tag="kh", name="kh")
            passes = (0, 1)
            NP = NCI // 2
            NMM = len(passes) * NP

            def mm(pk, m, n, lo, cp, k):
                nc.tensor.matmul(
                    pk, wk_hh[:, 2 * cp:2 * cp + 2, m * 128:(m + 1) * 128],
                    if2[:, 2 * cp:2 * cp + 2, lo, n * 288:(n + 1) * 288],
                    start=(k == 0), stop=(k == NMM - 1), perf_mode=DRow)

            if c_outer:
                # chunk-pair-outer over 4 open psums, with a caller-provided
                # emission (pair-0's Q) interleaved after cp 1 so the PE has
                # work while the tail of the image stream arrives
                pks = {}
                for m in range(2):
                    for n in range(2):
                        pks[(m, n)] = psum.tile([128, 288], f32, tag="ps512",
                                                name="ps512")
                k = 0
                for cp in range(NP):
                    if cp == 2 and c_outer is not True:
                        c_outer()
                    for lo in passes:
                        for m in range(2):
                            for n in range(2):
                                mm(pks[(m, n)], m, n, lo, cp, k)
                        k += 1
                for m in range(2):
                    for n in range(2):
                        nc.scalar.activation(
                            out=k_h[:, m, n * 288:(n + 1) * 288],
                            in_=pks[(m, n)], func=AF.Identity,
                            bias=bk_sb[:, h * 2 + m:h * 2 + m + 1], scale=RS)
            else:
                for m in range(2):
                    for n in range(2):
                        pk = psum.tile([128, 288], f32, tag="ps512", name="ps512")
                        k = 0
                        for lo in passes:
                            for cp in range(NP):
                                mm(pk, m, n, lo, cp, k)
                                k += 1
                        nc.scalar.activation(
                            out=k_h[:, m, n * 288:(n + 1) * 288], in_=pk,
                            func=AF.Identity,
                            bias=bk_sb[:, h * 2 + m:h * 2 + m + 1], scale=RS)
            S[i]["k"] = k_h

        def emit_V(i, c_outer=False):
            b, h = divmod(i, NH)
            if2 = TFI[b][1]
            wv2 = W[i][2]
            v_h = atp.tile([128, 5, HD], bf16, tag="vh", name="vh")
            # image is the stationary side here: V = (ih+il)^T wvh + ih^T wvl
            passes = ((0, 0), (0, 1), (1, 0))  # (image lo, wv lo)
            NP = NCI // 2
            NMM = len(passes) * NP

            def mm(pv, t, pt, ilo, wlo, cp, k):
                nc.tensor.matmul(
                    pv[:pt],
                    if2[:, 2 * cp:2 * cp + 2, ilo, t * 128:t * 128 + pt],
                    wv2[:, 2 * cp:2 * cp + 2, wlo, :],
                    start=(k == 0), stop=(k == NMM - 1), perf_mode=DRow)

            if c_outer:
                pvs = {}
                for t, pt in ITC:
                    pvs[t] = psum.tile([128, HD], f32, tag="ps512", name="ps512")
                k = 0
                for cp in range(NP):
                    for ilo, wlo in passes:
                        for t, pt in ITC:
                            mm(pvs[t], t, pt, ilo, wlo, cp, k)
                        k += 1
                for t, pt in ITC:
                    nc.vector.tensor_scalar(
                        out=v_h[:pt, t, :], in0=pvs[t][:pt], scalar1=RS,
                        scalar2=None, op0=ALU.mult)
            else:
                for t, pt in ITC:
                    pv = psum.tile([128, HD], f32, tag="ps512", name="ps512")
                    k = 0
                    for ilo, wlo in passes:
                        for cp in range(NP):
                            mm(pv, t, pt, ilo, wlo, cp, k)
                            k += 1
                    nc.vector.tensor_scalar(
                        out=v_h[:pt, t, :], in0=pv[:pt], scalar1=RS,
                        scalar2=None, op0=ALU.mult)
            S[i]["v"] = v_h

        def emit_scores(i):
            q_h, k_h = S[i]["q"], S[i]["k"]
            e_f = atp.tile([128, 5, LT], bf16, tag="ef", name="ef")
            e8_f = atp.tile([128, 4, LT], f8, tag="e8f", name="e8f")
            for t, pt in ITC:
                ps_s = psum.tile([128, LT], f32, tag="ps512", name="ps512")
                nc.tensor.matmul(
                    ps_s[:pt], k_h[:, :, t * 128:t * 128 + pt], q_h[:, :, :],
                    start=True, stop=True, perf_mode=DRow)
                nc.scalar.activation(
                    out=e_f[:pt, t, :], in_=ps_s[:pt], func=AF.Exp, scale=ISCALE)
                # fp8 copy of e (for the DoubleRow denominator matmul; the 576
                # keys average the 2^-4 noise away) on the otherwise-idle
                # GPSIMD engine; px keeps the bf16 e. The 64-token tail chunk
                # sums in bf16 directly so pd never waits on the last copy.
                if t < 4:
                    nc.gpsimd.tensor_copy(out=e8_f[:pt, t, :],
                                          in_=e_f[:pt, t, :])
            S[i]["e"] = e_f
            S[i]["e8"] = e8_f

        def emit_scores_V(i):
            # interleave each scores chunk with a V chunk: the PE blasts the 5
            # scores matmuls in ~0.5us but the exp Acts drain their PSUM banks
            # ~570ns apiece, so back-to-back scores->V stalls on bank recycle
            q_h, k_h = S[i]["q"], S[i]["k"]
            if2 = TFI[divmod(i, NH)[0]][1]
            wv_hh = W[i][2]
            e_f = atp.tile([128, 5, LT], bf16, tag="ef", name="ef")
            e8_f = atp.tile([128, 4, LT], f8, tag="e8f", name="e8f")
            v_h = atp.tile([128, 5, HD], bf16, tag="vh", name="vh")
            passes = (0, 1)   # V = (ih+il)^T wvh (single-fp8 V weights)
            NP = NCI // 2
            NMM = len(passes) * NP
            for t, pt in ITC:
                ps_s = psum.tile([128, LT], f32, tag="ps512", name="ps512")
                nc.tensor.matmul(
                    ps_s[:pt], k_h[:, :, t * 128:t * 128 + pt], q_h[:, :, :],
                    start=True, stop=True, perf_mode=DRow)
                nc.scalar.activation(
                    out=e_f[:pt, t, :], in_=ps_s[:pt], func=AF.Exp, scale=ISCALE)
                if t < 4:
                    nc.gpsimd.tensor_copy(out=e8_f[:pt, t, :],
                                          in_=e_f[:pt, t, :])
                pv = psum.tile([128, HD], f32, tag="ps512", name="ps512")
                k = 0
                for ilo in passes:
                    for cp in range(NP):
                        nc.tensor.matmul(
                            pv[:pt],
                            if2[:, 2 * cp:2 * cp + 2, ilo,
                                t * 128:t * 128 + pt],
                            wv_hh[:, 2 * cp:2 * cp + 2, :],
                            start=(k == 0), stop=(k == NMM - 1),
                            perf_mode=DRow)
                        k += 1
                nc.vector.tensor_scalar(
                    out=v_h[:pt, t, :], in0=pv[:pt], scalar1=RS,
                    scalar2=None, op0=ALU.mult)
            S[i]["e"] = e_f
            S[i]["e8"] = e8_f
            S[i]["v"] = v_h

        def emit_pd(i):
            e8_f = S[i]["e8"]
            pd2 = psd.tile([1, LT], f32, tag="psd", name="psd")
            for tp in range(2):
                nc.tensor.matmul(pd2[0:1, :], ones8_pd[:, :, 0:1],
                                 e8_f[:, 2 * tp:2 * tp + 2, :],
                                 start=(tp == 0), stop=False, perf_mode=DRow)
            nc.tensor.matmul(pd2[0:1, :], ones_col_b[:64],
                             S[i]["e"][:64, 4, :], start=False, stop=True)
            recip = atp.tile([1, LT], bf16, tag="recip", name="recip")
            with nc.allow_low_precision(reason="softmax recip feeds bf16 matmul"):
                nc.vector.reciprocal(out=recip, in_=pd2[0:1, :])
            S[i]["recip"] = recip

        def emit_pbc(i):
            # broadcast the softmax reciprocal across partitions on the
            # (mostly idle) GPSIMD instead of a PE ones-matmul + Act copy
            bcast = atp.tile([128, LT], bf16, tag="bcast", name="bcast")
            nc.gpsimd.partition_broadcast(bcast, S[i]["recip"])
            S[i]["bcast"] = bcast

        def emit_px(i):
            emit_px_mms(i)
            emit_px_mult(i)

        def emit_px_mms(i):
            # the attention-value matmuls depend only on v/e, not on the
            # softmax reciprocal: emitting them before pd gives the GPSIMD
            # e8 copies (pd's DoubleRow operands) ~2us more cover
            v_h, e_f = S[i]["v"], S[i]["e"]
            pxs = []
            for m in range(2):
                px = psum.tile([128, LT], f32, tag="ps512", name="ps512")
                for t, pt in ITC:
                    nc.tensor.matmul(
                        px, v_h[:pt, t, m * 128:(m + 1) * 128], e_f[:pt, t, :],
                        start=(t == 0), stop=(t == 4))
                pxs.append(px)
            S[i]["pxs"] = pxs

        def emit_px_mult(i):
            b, h = divmod(i, NH)
            bcast = S[i]["bcast"]
            for m in range(2):
                nc.vector.tensor_mul(out=r(XF[b][h * 2 + m]),
                                     in0=S[i]["pxs"][m], in1=bcast)
            S[i] = None  # release references

        def emit_rev(b, split_first=None, fuse_ff1=False):
            ofs = [ofp.tile([128, LT], f32, tag=f"of{m}", name=f"of{m}")
                   for m in range(NCD)]
            ofb = [ofp.tile([128, LT], bf16, tag=f"ofb{m}", name=f"ofb{m}")
                   for m in range(NCD)]
            NC16 = H // 128
            # fused FF1: emit each FF1 matmul lagged two chunks behind its
            # ofb writeback so the PE never waits on the Act/DVE queue; this
            # pulls FF1 inside rev's span and shortens the serial tail chain.
            # ph lives in the psd pool (same 2KB bank footprint as pstA/B).
            ph = psd.tile([128, LT], f32, tag="psd", name="psd") \
                if fuse_ff1 else None

            def ff1mm(m):
                nc.tensor.matmul(ph, w1_sb[:, m, :], ofb[m],
                                 start=(m == 0), stop=(m == NCD - 1))

            def mm(po, m, c, start, stop):
                nc.tensor.matmul(po, r(wr_sb[:, c, m * 128:(m + 1) * 128]),
                                 r(XF[b][c]), start=start, stop=stop)

            def writeback(m, po):
                if m < NCD - 1:
                    nc.scalar.activation(
                        out=r(ofs[m]), in_=po, func=AF.Identity,
                        bias=breff_sb[:, m:m + 1], scale=1.0)
                    nc.vector.tensor_scalar(
                        out=ofb[m], in0=po, scalar1=breff_sb[:, m:m + 1],
                        scalar2=None, op0=ALU.add)
                else:
                    # last chunk: ofb gates FF1's final matmul - put it on the
                    # idle Act queue, ofs on DVE
                    nc.scalar.activation(
                        out=ofb[m], in_=po, func=AF.Identity,
                        bias=breff_sb[:, m:m + 1], scale=1.0)
                    nc.vector.tensor_scalar(
                        out=r(ofs[m]), in0=po, scalar1=breff_sb[:, m:m + 1],
                        scalar2=None, op0=ALU.add)

            if split_first is not None:
                # overlap the pair-15 softmax tail and its xf writes with
                # partial accumulations of the first two output chunks
                po0 = psum.tile([128, LT], f32, tag="ps512", name="ps512")
                for c in range(NC16 - 2):
                    mm(po0, 0, c, c == 0, False)
                split_first()
                po1 = psum.tile([128, LT], f32, tag="ps512", name="ps512")
                for c in range(NC16 - 2):
                    mm(po1, 1, c, c == 0, False)
                for c in (NC16 - 2, NC16 - 1):
                    mm(po0, 0, c, False, c == NC16 - 1)
                writeback(0, po0)
                for c in (NC16 - 2, NC16 - 1):
                    mm(po1, 1, c, False, c == NC16 - 1)
                writeback(1, po1)
                first = 2
            else:
                first = 0
            for m in range(first, NCD):
                po = psum.tile([128, LT], f32, tag="ps512", name="ps512")
                for c in range(NC16):
                    mm(po, m, c, c == 0, c == NC16 - 1)
                writeback(m, po)
                if fuse_ff1 and m >= 2:
                    ff1mm(m - 2)
            if fuse_ff1:
                ff1mm(NCD - 2)
                ff1mm(NCD - 1)
            OF[b] = (ofs, ofb)
            return ph

        def emit_ff_stats(b, mid=None, ph=None):
            ofs, ofb = OF[b]
            i32 = mybir.dt.int32
            if ph is None:
                ph = psum.tile([128, LT], f32, tag="ps512", name="ps512")
                for c in range(NCD):
                    nc.tensor.matmul(ph, w1_sb[:, c, :], ofb[c],
                                     start=(c == 0), stop=(c == NCD - 1))
            h_sb = p5p.tile([128, LT], bf16, tag="hsb", name="hsb")
            nc.scalar.activation(out=h_sb, in_=ph, func=AF.Relu, bias=b1_sb,
                                 scale=1.0)
            if mid is not None:
                mid()   # PE work to cover the FF1 -> relu -> FF2 round trip
            pstA = psd.tile([16, LT], f32, tag="psd", name="psd")
            pstB = psd.tile([16, LT], f32, tag="psd", name="psd")

            def pf_mm(m):
                pf = psum.tile([128, LT], f32, tag="ps512", name="ps512")
                if m % 2 == 0:
                    # DVE path: fused ofs[m] = (pf + b2[m]) + ofs[m]
                    nc.tensor.matmul(pf, w2_sb[:, m * 128:(m + 1) * 128], h_sb)
                    nc.vector.scalar_tensor_tensor(
                        out=r(ofs[m]), in0=pf, scalar=b2_sb[:, m:m + 1],
                        in1=ofs[m], op0=ALU.add, op1=ALU.add)
                    nc.scalar.activation(out=ofb[m], in_=ofs[m],
                                         func=AF.Square, scale=1.0)
                else:
                    # PE path: residual via identity-accumulate, Act writeback
                    nc.tensor.matmul(pf, w2_sb[:, m * 128:(m + 1) * 128], h_sb,
                                     start=True, stop=False)
                    nc.tensor.matmul(pf, r(ident_r), r(ofs[m]), start=False,
                                     stop=True)
                    nc.scalar.activation(out=r(ofs[m]), in_=pf,
                                         func=AF.Identity,
                                         bias=b2_sb[:, m:m + 1], scale=1.0)
                    nc.vector.tensor_mul(out=ofb[m], in0=ofs[m], in1=ofs[m])

            def stA(m):
                nc.tensor.matmul(pstA, r(ones16_f), r(ofs[m]),
                                 start=(m == 0), stop=(m == NCD - 1))

            def stB(m):
                nc.tensor.matmul(pstB, ones16_b, ofb[m],
                                 start=(m == 0), stop=(m == NCD - 1))

            # interleave stats accumulation behind the pf matmuls so PE never
            # waits on the stt/sq chains
            pf_mm(0)
            pf_mm(1)
            stA(0)
            pf_mm(2)
            stA(1)
            stB(0)
            pf_mm(3)
            stA(2)
            stB(1)
            pf_mm(4)
            stA(3)
            stB(2)
            pf_mm(5)
            stA(4)
            stB(3)
            stA(5)
            stB(4)
            stB(5)

            srowA = ofp.tile([16, LT], f32, tag="srowA", name="srowA")
            srowB = ofp.tile([16, LT], f32, tag="srowB", name="srowB")
            nc.scalar.activation(out=r(srowA), in_=pstA, func=AF.Copy)
            nc.scalar.activation(out=r(srowB), in_=pstB, func=AF.Copy)
            mq4 = p5p.tile([128, NT, 32], f32, tag="mq4", name="mq4")
            ptc = psum.tile([128, NT, 32], f32, tag="ps512", name="ps512")
            for t in range(NT):
                nc.tensor.transpose(r(ptc[:, t, 0:16]),
                                    r(srowA[:, t * 128:(t + 1) * 128]),
                                    r(ident_r[:16, :16]))
                nc.tensor.transpose(r(ptc[:, t, 16:32]),
                                    r(srowB[:, t * 128:(t + 1) * 128]),
                                    r(ident_r[:16, :16]))
            nc.vector.tensor_scalar(out=mq4, in0=ptc, scalar1=1.0 / DT,
                                    scalar2=None, op0=ALU.mult)
            mu4 = mq4[:, :, 0]
            q4 = mq4[:, :, 16]
            musq = p5p.tile([128, NT], f32, tag="musq4", name="musq4")
            nc.vector.tensor_mul(out=musq, in0=mu4, in1=mu4)
            v4 = p5p.tile([128, NT], f32, tag="v4", name="v4")
            nc.vector.tensor_sub(out=v4, in0=q4, in1=musq)
            # rstd = rsqrt(v4): quake bit-trick seed + Newton steps, DVE only
            y4 = p5p.tile([128, NT], f32, tag="y4", name="y4")
            sh4 = p5p.tile([128, NT], f32, tag="sh4", name="sh4")
            nc.vector.tensor_scalar(
                out=sh4.bitcast(i32), in0=v4.bitcast(i32), scalar1=1,
                scalar2=None, op0=ALU.logical_shift_right)
            nc.vector.tensor_sub(out=y4.bitcast(i32), in0=magic4.bitcast(i32),
                                 in1=sh4.bitcast(i32))
            t14 = p5p.tile([128, NT], f32, tag="t14", name="t14")
            for _ in range(1):
                nc.vector.tensor_mul(out=t14, in0=y4, in1=y4)
                nc.vector.tensor_mul(out=t14, in0=t14, in1=v4)
                nc.vector.tensor_scalar(out=t14, in0=t14, scalar1=-0.5,
                                        scalar2=1.5, op0=ALU.mult, op1=ALU.add)
                nc.vector.tensor_mul(out=y4, in0=y4, in1=t14)
            nmr4 = p5p.tile([128, NT], f32, tag="nmr4", name="nmr4")
            nc.vector.scalar_tensor_tensor(
                out=nmr4, in0=mu4, scalar=-1.0, op0=ALU.mult, in1=y4,
                op1=ALU.mult)
            stats = [(y4[:, t:t + 1], nmr4[:, t:t + 1]) for t in range(NT)]
            OF[b] = (ofs, stats)

        def emit_ln_chunk(b, t):
            ofs, stats = OF[b]
            rstd, nmr = stats[t]
            # batch 1 (the tail) gets a distinct y tile per chunk so the four
            # normalize->store chains pipeline instead of serializing on tile
            # reuse; batch 0's chunks are spread across pairs and can share
            ytag = f"y{t % 3}" if b == 1 else "y0"
            ypool = ofp if b == 1 else p5p   # ofp is bufs=1: no double alloc
            y = ypool.tile([128, DT], f32, tag=ytag, name=ytag)
            for half in range(2):
                ptr_ = psum.tile([128, 384], f32, tag="ps512", name="ps512")
                for j in range(3):
                    c = half * 3 + j
                    nc.tensor.transpose(
                        r(ptr_[:, j * 128:(j + 1) * 128]),
                        r(ofs[c][:, t * 128:(t + 1) * 128]), r(ident_r))
                dst = y[:, half * 384:(half + 1) * 384]
                if half == 0:
                    nc.scalar.activation(out=dst, in_=ptr_, func=AF.Identity,
                                         bias=nmr, scale=rstd)
                else:
                    nc.vector.tensor_scalar(
                        out=dst, in0=ptr_, scalar1=rstd, scalar2=nmr,
                        op0=ALU.mult, op1=ALU.add)
                if apply_gamma:
                    nc.vector.tensor_mul(out=dst, in0=dst,
                                         in1=gam_sb[:, half * 384:(half + 1) * 384])
                    nc.vector.tensor_add(out=dst, in0=dst,
                                         in1=bet_sb[:, half * 384:(half + 1) * 384])
            # one whole-chunk store per (b, t) on the SP queue: fewest
            # HWDGE slots and DMA-sem edges
            nc.sync.dma_start(out=out[b, t * 128:(t + 1) * 128, :], in_=y)

        def load_text(b, eng=None):
            tf2 = tfp.tile([128, NCD, 2, LT], f8, tag="textf2", name="textf2")
            (eng or nc.sync).dma_start(out=tf2, in_=t2p[b])
            return tf2

        def alloc_image():
            return tfp.tile([128, NCI, 2, LI], f8, tag="imgf2", name="imgf2")

        def load_image_part(if2, b, c0, c1, eng=None):
            (eng or nc.sync).dma_start(
                out=if2[:, c0:c1], in_=i2p[b, :, c0:c1])

        def load_image(b, eng=None):
            if2 = alloc_image()
            load_image_part(if2, b, 0, 8, eng)
            return if2

        # ---------------- emission schedule ----------------
        # startup: the HWDGE serializes ~625ns per DMA instruction, each DMA
        # dependency edge pays a 900ns sem-prop, and the bus runs ~330GB/s.
        # Order the queue by pair-0's fine-grained consumption and interleave
        # K and Q chunk-group matmuls with the piece arrivals.
        wk_h0h = hwp.tile([128, NCI, HD], f8, tag="wkhh", name="wkhh")
        nc.sync.dma_start(out=wk_h0h, in_=wkph[:, 0])
        if0 = alloc_image()
        load_image_part(if0, 0, 0, 2)   # image chunks 0-1 (h+l)
        wq_h0h = hwp.tile([128, NCD, HD], f8, tag="wqhh", name="wqhh")
        nc.sync.dma_start(out=wq_h0h, in_=wqph[:, 0])
        tfs0 = tfp.tile([128, NCD, 2, LT], f8, tag="textf2", name="textf2")
        nc.sync.dma_start(out=tfs0[:, 0:4], in_=t2p[0, :, 0:4])
        load_image_part(if0, 0, 2, 4)
        # biases mid-queue: tiny transfers, but each costs a 625ns shared
        # HWDGE slot; at the queue head they delay the first K matmul
        nc.scalar.dma_start(out=bk_sb, in_=bkp)
        nc.scalar.dma_start(out=bq_sb, in_=bqp)
        nc.sync.dma_start(out=tfs0[:, 4:6], in_=t2p[0, :, 4:6])
        load_image_part(if0, 0, 4, 8)
        wv_h0h = hwp.tile([128, NCI, HD], f8, tag="wvhh", name="wvhh")
        nc.sync.dma_start(out=wv_h0h, in_=wvph[:, 0])
        TFI[0] = (tfs0, if0)
        W[0] = (wq_h0h, wk_h0h, wv_h0h)
        alloc_xf(0)

        # ---- pair 0: K/Q chunk-groups emitted in DMA-arrival order ----
        S[0] = {}
        pks0 = {}
        for m in range(2):
            for n in range(2):
                pks0[(m, n)] = psum.tile([128, 288], f32, tag="ps512",
                                         name="ps512")

        def k0_cp(cp, first, last):
            for lo in range(2):
                for m in range(2):
                    for n in range(2):
                        nc.tensor.matmul(
                            pks0[(m, n)],
                            wk_h0h[:, 2 * cp:2 * cp + 2, m * 128:(m + 1) * 128],
                            if0[:, 2 * cp:2 * cp + 2, lo,
                                n * 288:(n + 1) * 288],
                            start=(first and lo == 0), stop=(last and lo == 1),
                            perf_mode=DRow)

        def q0_cp(cp, first, last):
            for m in range(2):
                for lo in range(2):
                    nc.tensor.matmul(
                        pqs0[m],
                        wq_h0h[:, 2 * cp:2 * cp + 2, m * 128:(m + 1) * 128],
                        tfs0[:, 2 * cp:2 * cp + 2, lo, :],
                        start=(first and lo == 0), stop=(last and lo == 1),
                        perf_mode=DRow)

        k0_cp(0, True, False)
        pqs0 = [psum.tile([128, LT], f32, tag="ps512", name="ps512")
                for _ in range(2)]
        q0_cp(0, True, False)
        k0_cp(1, False, False)
        q0_cp(1, False, False)
        q0_cp(2, False, True)
        q_h0 = atp.tile([128, 2, LT], f8, tag="qh", name="qh")
        for m in range(2):
            nc.scalar.activation(out=q_h0[:, m, :], in_=pqs0[m],
                                 func=AF.Identity, bias=bq_sb[:, m:m + 1],
                                 scale=RS)
        S[0]["q"] = q_h0
        k0_cp(2, False, False)
        k0_cp(3, False, True)
        k_h0 = atp.tile([128, 2, LI], f8, tag="kh", name="kh")
        for m in range(2):
            for n in range(2):
                nc.scalar.activation(
                    out=k_h0[:, m, n * 288:(n + 1) * 288], in_=pks0[(m, n)],
                    func=AF.Identity, bias=bk_sb[:, m:m + 1], scale=RS)
        S[0]["k"] = k_h0
        emit_scores_V(0)
        load_weights(1)
        load_weights(2)

        prev = 0
        for i in range(1, NPAIR - 1):
            if i + 2 <= NPAIR - 1:
                load_weights(i + 2)   # 2-pair prefetch distance (hwp bufs=3)
            if i == 2:
                load_consts_late()
            if i == 5:
                TFI1_T = load_text(1)
            if i == 6:
                IF1 = alloc_image()
                load_image_part(IF1, 1, 0, 4)
            if i == 7:
                load_image_part(IF1, 1, 4, 8)
                TFI[1] = (TFI1_T, IF1)
            if 1 <= i <= 8:
                load_wr_chunk(i - 1)
            emit_Q(i)
            if i == 9:
                # fill the FF1->relu->FF2 round trip of batch 0 with
                # pair-9's K matmuls
                emit_ff_stats(0, mid=lambda: emit_K(i))
            else:
                emit_K(i)
            # pd after K: its DoubleRow operands are the GPSIMD e8 copies of
            # the previous pair, which finish ~3.4us after the last exp
            emit_pd(prev)
            emit_pbc(prev)
            emit_px(prev)
            emit_scores_V(i)
            if i == 8:
                emit_rev(0)
                alloc_xf(1)
            elif 10 <= i <= 13:
                emit_ln_chunk(0, i - 10)
            prev = i
        # ---- block 15: scores before V so exp(15) finishes during V;
        # reversion's first group splits around the pair-15 softmax tail ----
        i = NPAIR - 1
        emit_Q(i)
        emit_K(i)
        emit_pd(prev)
        emit_pbc(prev)
        emit_px(prev)
        emit_scores_V(i)
        emit_pd(i)

        def _tail15():
            emit_pbc(i)
            emit_px(i)

        emit_rev(1, split_first=_tail15)
        emit_ff_stats(1)
        for t in range(NT):
            emit_ln_chunk(1, t)

    nc.compile()
    _BUILD_CACHE[key] = nc
    return nc


def _prep_in_maps(inputs):
    import ml_dtypes

    F8 = ml_dtypes.float8_e4m3

    def bf(x):
        return np.ascontiguousarray(np.asarray(x, dtype=np.float32).astype(
            ml_dtypes.bfloat16))

    def f32c(x):
        return np.ascontiguousarray(np.asarray(x, dtype=np.float32))

    def split8(x):
        x = np.asarray(x, np.float32)
        xh = x.astype(F8)
        xl = (x - xh.astype(np.float32)).astype(F8)
        return xh, xl

    def featmajor(x, L, D):
        # [B_TOTAL, L, D] -> [B_TOTAL, 128, D//128, L]
        return np.ascontiguousarray(
            np.asarray(x).reshape(-1, L, D // 128, 128).transpose(0, 3, 2, 1))

    def headmajor(w, din):
        # [din, H] -> [128, NH, din//128, HD] (per-partition contiguous runs)
        return np.ascontiguousarray(
            np.asarray(w).reshape(din // 128, 128, NH, HD).transpose(1, 2, 0, 3))

    th, tl = split8(inputs["text"])
    ih, il = split8(inputs["image"])
    t2 = np.ascontiguousarray(np.stack(
        (featmajor(th, LT, DT), featmajor(tl, LT, DT)), axis=3))
    i2 = np.ascontiguousarray(np.stack(
        (featmajor(ih, LI, DI), featmajor(il, LI, DI)), axis=3))
    wqh = (np.asarray(inputs["wq"], np.float32) * WS).astype(F8)
    wkh = (np.asarray(inputs["wk"], np.float32) * WS).astype(F8)
    wvh = (np.asarray(inputs["wv"], np.float32) * WS).astype(F8)
    wr = np.asarray(inputs["wr"], dtype=np.float64)
    bv = np.asarray(inputs["bv"], dtype=np.float64)
    br = np.asarray(inputs["br"], dtype=np.float64)
    breff = (br + bv @ wr).astype(np.float32)
    w1 = bf(inputs["w1"])
    gamma = f32c(inputs["gamma"])
    beta = f32c(inputs["beta"])

    shared = {
        "wqph": headmajor(wqh, DT),
        "wkph": headmajor(wkh, DI),
        "wvph": headmajor(wvh, DI),
        "wrp": np.ascontiguousarray(
            f32c(inputs["wr"]).reshape(H // 128, 128, DT).transpose(1, 0, 2)),
        "w1p": np.ascontiguousarray(
            w1.reshape(NCD, 128, FF).transpose(1, 0, 2)),
        "w2": bf(inputs["w2"]),
        "bqp": np.ascontiguousarray(
            f32c(inputs["bq"]).reshape(H // 128, 128).T),
        "bkp": np.ascontiguousarray(
            f32c(inputs["bk"]).reshape(H // 128, 128).T),
        "b1p": np.ascontiguousarray(f32c(inputs["b1"]).reshape(128, 1)),
        "b2p": np.ascontiguousarray(f32c(inputs["b2"]).reshape(NCD, 128).T),
        "breffp": np.ascontiguousarray(breff.reshape(NCD, 128).T),
        "gamp": np.ascontiguousarray(np.broadcast_to(gamma, (128, DT))),
        "betp": np.ascontiguousarray(np.broadcast_to(beta, (128, DT))),
    }
    in_maps = []
    for c in range(N_CORES):
        m = dict(shared)
        m["t2p"] = t2[c * B:(c + 1) * B]
        m["i2p"] = i2[c * B:(c + 1) * B]
        in_maps.append(m)
    return in_maps


def _needs_gamma(inputs):
    g = np.asarray(inputs["gamma"], dtype=np.float32)
    b = np.asarray(inputs["beta"], dtype=np.float32)
    return not (np.all(g == 1.0) and np.all(b == 0.0))


def kernel(**inputs) -> np.ndarray:
    _ensure_import_path()
    from concourse.bass_utils import run_bass_kernel_spmd

    nc = build_module(apply_gamma=_needs_gamma(inputs))
    in_maps = _prep_in_maps(inputs)
    res = run_bass_kernel_spmd(nc, in_maps, core_ids=list(range(N_CORES)))
    return np.concatenate([res.results[c]["out"] for c in range(N_CORES)], axis=0)
